# revision 1
# baseline (speedup 1.0000x reference)
"""Trainium2 Bass kernel for the Haar-mask MLP (histogram_binning).

Every Haar interval edge is a multiple of 2^-10, so the reference's masks --
and therefore the entire MLP output -- depend only on u = floor(t * 1024)
(exact in fp32).  The network collapses to a 1024x3 lookup table computed on
host from the tiny weights; the device work is: stream t, compute u, gather
LUT[u], stream out.

Gather engine: SWDGE dma_gather.  Q7 core-pairs (one per queue, 4 queues)
generate SDMA descriptors (16 gather packets each, 128-descriptor ring) and
the SDMA engines pull 16-byte LUT rows from 256B-strided HBM rows.  Measured
~2.6 ns/element aggregate -- 2.2x the old gpsimd indirect_copy ucode path
(~3.5 ns/elem) once instruction gen, ring stalls and tail are accounted.
1024-idx instructions (65 ring descriptors each) balance per-instruction
Q7 generation overhead against ring-full stall granularity.

Raw Block structure (no TileContext): avoids the per-DMA InstIncSwdgeSem
bookkeeping (~1.3 us each) the tile framework inserts.

Layouts per core (16384 elements, j = element ordinal):
  t_d  [128, 1024] f32: t[j] at partition 16g + j%16 (all 8 groups g),
       column j//16 -- the wrapped index layout whose per-group replicas the
       dma_gather tx cores read.  DMA'd in 4 column-quarters so the index
       chain starts after the first quarter lands.
  idx  uint16 [128, 1024] computed on DVE in 160-column chunks (the final
       chunk is 224 columns: narrow trailing uint16 chunks mis-write).
  dst/out [128, 128, 4] f32: element j at [j%128, j//128]; 4th word pad.
"""

import numpy as np
from contextlib import ExitStack

from concourse import bacc, mybir
from concourse.bass_utils import run_bass_kernel_spmd
from concourse.library_config import mlp as mlp_lib

N_CORES = 8
B, T, F = 16, 8192, 3
N = B * T                    # 131072 total elements
NPC = N // N_CORES           # 16384 per neuron core
P = 128
S = NPC // P                 # 128 slots per partition
NBINS = 1024
ROWW = 64                    # LUT row stride: 64 f32 = 256 B (SDMA stride unit)
GE = 4                       # gathered f32 per element (16 B payload)
NQ = 4                       # SWDGE queues (ucode max)
NG = 16                      # dma_gather instructions (1024 idx each)
COLS = NPC // 16             # 1024 idx columns
CHB = [(0, 160), (160, 320), (320, 480), (480, 640), (640, 800), (800, COLS)]

IMPL = "dg"
RUN_KWARGS = {}
LAST_RESULTS = None
_CACHE = {}


def _build_lut(W1, b1, W2, b2, W3, b3):
    """MLP output for each of the 1024 half-interval bins, fp32 math."""
    u = np.arange(NBINS)
    acc = np.zeros((NBINS, W1.shape[1]), np.float32)
    for j in range(10):
        k = u >> (10 - j)                       # floor(t * 2^j) for t in bin u
        idx = (1 << j) - 1 + k                  # level-j block offset + k
        sign = np.where((u >> (9 - j)) & 1 == 0, np.float32(1), np.float32(-1))
        acc = acc + sign[:, None] * W1[idx]
    h = np.maximum(acc + b1, np.float32(0))
    h = np.maximum(h @ W2 + b2, np.float32(0))
    return (h @ W3 + b3).astype(np.float32)     # (1024, 3)


def _dma_gather_raw(gp, out_ap, in_ap, idxs_ap, num_idxs, elem_size, elem_step,
                    queue_num):
    """gpsimd.dma_gather minus the elem_size_bytes%256 assert (non-transpose
    HBM path: only the row STRIDE must be a 256B multiple, not the payload).
    Verified on hardware with 16B payloads."""
    _in_ap = gp.lower_ap_dma(in_ap, for_custom_bir_dma=True)
    return gp.add_instruction(mybir.InstDMAGatherAnt(
        name=gp.bass.get_next_instruction_name(),
        ins=[*_in_ap, gp.lower_ap(idxs_ap),
             gp.lower_val_access(gp.to_reg(num_idxs))],
        outs=[gp.lower_ap(out_ap)],
        transpose=False, num_idxs=num_idxs, elem_size=elem_size,
        stride_bytes_256=elem_step * 4 // 256, gen_mode=0, single_packet=True,
        queue_num=queue_num, sbuf_tokens_per_rank=0, sbuf_free_dim_per_rank=0,
        sbuf_free_dim_pad_per_rank=0, sbuf_byte_offset=0))


def _build_nc():
    nc = bacc.Bacc("TRN2", target_bir_lowering=False, debug=False,
                   enable_asserts=False, num_devices=N_CORES,
                   num_swdge_queues=NQ)
    f32 = mybir.dt.float32
    t_d = nc.dram_tensor("t", [P, COLS], f32, kind="ExternalInput")
    lut_d = nc.dram_tensor("lut", [NBINS, ROWW], f32, kind="ExternalInput")
    out_d = nc.dram_tensor("out", [P, S, GE], f32, kind="ExternalOutput")

    ipg = NPC // NG                              # indices per gather: 512
    cpg = COLS // NG                             # idx columns per gather: 16
    spg = S // NG                                # dst slots per gather: 4
    gpr = NG // NQ                               # rounds: 8
    tq = COLS // 4                               # t DMA column quarter

    with nc.Block() as block, ExitStack() as ctx:
        sb = lambda name, shape, dt: ctx.enter_context(
            nc.sbuf_tensor(name, shape, dt))
        sem = lambda name: ctx.enter_context(nc.semaphore(name))
        t_sb = sb("t_sb", [P, COLS], f32)
        uf = sb("uf", [P, COLS], f32)
        ii = sb("ii", [P, COLS], mybir.dt.int32)
        fb = sb("fb", [P, COLS], f32)
        adj = sb("adj", [P, COLS], f32)
        idx = sb("idx", [P, COLS], mybir.dt.uint16)
        dst = sb("dst", [P, S, GE], f32)
        ioa, iob, vs, ou = sem("ioa"), sem("iob"), sem("vs"), sem("ou")
        qsems = [sem(f"q{q}") for q in range(NQ)]

        @block.sync
        def _(s):
            s.dma_start(t_sb[:, 0 * tq:1 * tq], t_d[:, 0 * tq:1 * tq]
                        ).then_inc(ioa, 16)
            s.dma_start(t_sb[:, 2 * tq:3 * tq], t_d[:, 2 * tq:3 * tq]
                        ).then_inc(ioa, 16)

        @block.scalar
        def _(s):
            s.dma_start(t_sb[:, 1 * tq:2 * tq], t_d[:, 1 * tq:2 * tq]
                        ).then_inc(iob, 16)
            s.dma_start(t_sb[:, 3 * tq:4 * tq], t_d[:, 3 * tq:4 * tq]
                        ).then_inc(iob, 16)

        @block.vector
        def _(v):
            # quarter q0,q2 land via sync (ioa), q1,q3 via scalar (iob)
            need_a = {1: 16, 2: 16, 3: 32, 4: 32}
            need_b = {1: 0, 2: 16, 3: 16, 4: 32}
            da = db = 0
            for c0, c1 in CHB:
                nq = (c1 + tq - 1) // tq
                if need_a[nq] > da:
                    v.wait_ge(ioa, need_a[nq]); da = need_a[nq]
                if need_b[nq] > db:
                    v.wait_ge(iob, need_b[nq]); db = need_b[nq]
                sl = slice(c0, c1)
                # exact floor(t*1024): round-to-int (any rounding mode), then
                # subtract 1 wherever the rounded value exceeds the true value
                v.tensor_scalar(uf[:, sl], t_sb[:, sl], 1024.0, None,
                                mybir.AluOpType.mult)
                v.tensor_copy(ii[:, sl], uf[:, sl])
                v.tensor_copy(fb[:, sl], ii[:, sl])
                v.tensor_tensor(adj[:, sl], fb[:, sl], uf[:, sl],
                                mybir.AluOpType.is_gt)
                v.tensor_sub(fb[:, sl], fb[:, sl], adj[:, sl])
                v.tensor_scalar(idx[:, sl], fb[:, sl], 1023.0, 0.0,
                                mybir.AluOpType.min,
                                mybir.AluOpType.max).then_inc(vs, 1)

        @block.gpsimd
        def _(gp):
            gp.load_library(mlp_lib)
            for k in range(NG):
                # chain chunk covering idx columns [k*cpg, (k+1)*cpg)
                end = (k + 1) * cpg
                need = next(i + 1 for i, (c0, c1) in enumerate(CHB)
                            if c1 >= end)
                gp.wait_ge(vs, need)
                _dma_gather_raw(
                    gp, dst[:, k * spg:(k + 1) * spg, :], lut_d[:, 0:GE],
                    idx[:, k * cpg:(k + 1) * cpg], ipg, GE, ROWW,
                    k % NQ).then_inc(qsems[k % NQ], 16)

        @block.sync
        def _(s):
            for r in range(gpr):
                for q in range(NQ):
                    s.wait_ge(qsems[q], 16 * (r + 1))
                sl = slice(r * spg * NQ, (r + 1) * spg * NQ)
                s.dma_start(out_d.ap()[:, sl, :], dst[:, sl, :]).then_inc(ou, 16)
            s.wait_ge(ou, 16 * gpr)
    nc.compile()
    return nc


def _host_inputs(t, lut):
    tf = np.ascontiguousarray(np.asarray(t, np.float32)).reshape(-1)
    # wrapped layout: element j -> partition j%16, column j//16, x8 groups
    tw = tf.reshape(N_CORES, COLS, 16).transpose(0, 2, 1)       # [m, 16, cols]
    tperm = np.tile(tw, (1, 8, 1))                              # [m, 128, cols]
    lutp = np.zeros((NBINS, ROWW), np.float32)
    lutp[:, :F] = lut
    return tperm, lutp


def kernel(t, W1, b1, W2, b2, W3, b3):
    global LAST_RESULTS
    key = ("nc", IMPL)
    if key not in _CACHE:
        _CACHE[key] = _build_nc()
    nc = _CACHE[key]

    lut = _build_lut(np.asarray(W1, np.float32), np.asarray(b1, np.float32),
                     np.asarray(W2, np.float32), np.asarray(b2, np.float32),
                     np.asarray(W3, np.float32), np.asarray(b3, np.float32))
    tperm, lutp = _host_inputs(t, lut)
    in_maps = [{"t": np.ascontiguousarray(tperm[m]), "lut": lutp}
               for m in range(N_CORES)]

    res = run_bass_kernel_spmd(nc, in_maps, list(range(N_CORES)), **RUN_KWARGS)
    LAST_RESULTS = res
    # out[p, s] = element s*128 + p; 4th gathered word is pad
    outs = [res.results[m]["out"][:, :, :F].transpose(1, 0, 2).reshape(NPC, F)
            for m in range(N_CORES)]
    return np.concatenate(outs, axis=0).reshape(B, T, F).astype(np.float32)



# revision 3
# speedup vs baseline: 1.6189x; 1.6189x over previous
"""One-hot / TensorEngine LUT kernel for the Haar-mask MLP (histogram_binning).

Every Haar edge is a multiple of 2^-10, so the reference collapses to
out[e] = LUT[u], u = floor(1024*t) (LUT (1024,3) computed on host from the
tiny MLP weights).  Split u = 64*a + b and evaluate the lookup with compute
engines instead of per-element DMA gathers (the gather path is Q7
descriptor-rate bound at ~4.7 ns/element):

  P[e, f*16+a] = sum_b ohB[b, e] * M2[b, f*16+a]   (TensorE, contraction b)
  out[e, f]    = sum_a ohA[e, a] * P[e, f*16+a]    (DVE mask + tree reduce)

Per core (E = 16384, element e = p*128 + c; partition groups g = p>>6):
  1. DVE int chain -> lo = u & 63 (uint8), and a second chain on a
     host-transposed copy of t -> hi_t[c, p] = u >> 6 (fp16).
  2. lo replicated across the 64 b-partitions of each group via a DRAM
     bounce: store lo (8KB/group), then 4 staggered broadcast loads per
     group (leading stride-0 AP dim on the DRAM source).  uint8 halves the
     HBM re-read traffic; sems arrive progressively to pipeline the rest.
  3. ohb[(g,b), m] = is_equal(lo_rep, b)  [tensor_scalar, per-partition
     scalar] -- the transposed one-hot, directly in stationary orientation.
  4. TensorE: 64 paired matmuls: stationary = ohb[:, 128j:128j+128] (g0 rows
     0-63 + g1 rows 64-127), moving = block-diag(M2g0, M2g1) [128, 96] fp16
     -> PSUM [c, 96]: cols 0:48 tile (g0,j), 48:96 tile (g1,j).  One matmul
     per PSUM bank (two matmuls per bank hangs the HW); 4 tensors x 2 banks
     round-robin.
  5. ACT copies PSUM -> psb fp16 [c, pos2, f, a] (pos2 = 2j+g); DVE
     multiplies by ohA[c, pos2, a] = (hi_t == a) and tree-reduces over a
     in shrinking quarters; gpsimd casts fp16 -> fp32 on the out DMA.
Host: pure layout transforms (t || t^T, block-diag LUT table) + unpermute.
"""

import numpy as np
from contextlib import ExitStack

from concourse import bacc, mybir
from concourse.bass_utils import run_bass_kernel_spmd

N_CORES = 8
B, T, F = 16, 8192, 3
N = B * T
NPC = N // N_CORES            # 16384 per core
P = 128
C = NPC // P                  # 128 columns
NBINS = 1024
A = 16                        # hi one-hot width
Bw = 64                       # lo one-hot width (contraction)
NG = 2                        # partition groups of 64
GSZ = NPC // NG               # 8192 elements per group
NQ = 4                        # u_rep / ohB chunks
LCH = [(0, 1024), (1024, 3072), (3072, 5632), (5632, 8192)]
MQS = [(0, 40), (40, 80), (80, 112), (112, 128)]  # mult/tree/out quarters
NT = 128                      # PE tiles (one per source partition)
BCH = 2                       # paired matmuls per PE/ACT chunk (1 bank each)
NCH = NT // (2 * BCH)         # 32 chunks of 2 paired matmuls
MCH = 32                      # tiles per DVE mult chunk
KW = 3 * A                    # 48 psum columns per tile
KW2 = 2 * KW                  # 96 columns per paired matmul

# position pos2 = 2*jl + g (paired matmuls) -> source partition 64*g + jl
ORDER = [64 * (pos % 2) + pos // 2 for pos in range(NT)]

IMPL = "oh"
RUN_KWARGS = {}
LAST_RESULTS = None
_CACHE = {}

f32 = mybir.dt.float32
f16 = mybir.dt.float16
i32 = mybir.dt.int32
i16 = mybir.dt.int16
u8 = mybir.dt.uint8
Alu = mybir.AluOpType


def _build_lut(W1, b1, W2, b2, W3, b3):
    u = np.arange(NBINS)
    acc = np.zeros((NBINS, W1.shape[1]), np.float32)
    for j in range(10):
        k = u >> (10 - j)
        idx = (1 << j) - 1 + k
        sign = np.where((u >> (9 - j)) & 1 == 0, np.float32(1), np.float32(-1))
        acc = acc + sign[:, None] * W1[idx]
    h = np.maximum(acc + b1, np.float32(0))
    h = np.maximum(h @ W2 + b2, np.float32(0))
    return (h @ W3 + b3).astype(np.float32)     # (1024, 3)


def _build_nc():
    nc = bacc.Bacc("TRN2", target_bir_lowering=False, debug=False,
                   enable_asserts=False, num_devices=N_CORES)
    t_d = nc.dram_tensor("t", [P, 2 * C], f32, kind="ExternalInput")
    m2_d = nc.dram_tensor("m2", [P, KW2], f16, kind="ExternalInput")
    bv_d = nc.dram_tensor("bvec", [P, 1], f32, kind="ExternalInput")
    out_d = nc.dram_tensor("out", [P, NT, F], f32, kind="ExternalOutput")
    uscr_d = nc.dram_tensor("uscr", [NPC], u8, kind="Internal")

    with nc.Block() as block, ExitStack() as ctx:
        sb = lambda name, shape, dt: ctx.enter_context(
            nc.sbuf_tensor(name, shape, dt))
        sem = lambda name: ctx.enter_context(nc.semaphore(name))
        t_sb = sb("t_sb", [P, 2 * C], f32)
        m2 = sb("m2_sb", [P, KW2], f16)
        bvec = sb("bvec_sb", [P, 1], f32)
        x = sb("x", [P, C], f32)        # 1024*t
        iw = sb("iw", [P, C], i32)      # round(1024*t)
        xf = sb("xf", [P, C], f32)      # float(iw)
        adj = sb("adj", [P, C], i32)    # xf > x
        u_i = sb("u_i", [P, C], i32)
        lo32 = sb("lo32", [P, C], i32)
        lo16 = sb("lo16", [P, C], u8)
        hi_i = sb("hi_i", [P, C], i32)
        hi_t = sb("hi_t", [P, C], f16)          # hi from transposed t
        iota_a = sb("iota_a", [P, C, A], f16)   # value = a
        u_rep = sb("u_rep", [P, GSZ], u8)
        ohb = sb("ohb", [P, GSZ], f16)
        oha = sb("oha", [P, C, A], f16)
        psb = sb("psb", [P, NT, F, A], f16)
        y0 = sb("y0", [P, NT, F, 8], f16)
        y1 = sb("y1", [P, NT, F, 4], f16)
        y2 = sb("y2", [P, NT, F, 2], f16)
        yf = sb("yf", [P, NT, F], f16)
        psum = [ctx.enter_context(nc.psum_tensor(f"ps{k}", [P, BCH, 512], f32))
                for k in range(4)]

        s_warm = sem("s_warm")      # DVE warmup memset done
        s_in = sem("s_in")          # t DMA done
        s_in2 = sem("s_in2")        # m2 / bvec DMA done
        s_uscr = sem("s_uscr")      # lo bounced to DRAM (g0)
        s_uscr2 = sem("s_uscr2")    # lo bounced to DRAM (g1)
        s_chain = sem("s_chain")    # lo16 ready
        s_urep = sem("s_urep")      # u_rep DMA chunks (sync queue)
        s_urep2 = sem("s_urep2")    # u_rep DMA chunks (scalar queue)
        s_gp = sem("s_gp")          # gpsimd hi_rep done
        s_ohb = sem("s_ohb")        # ohB chunks done
        s_pe = sem("s_pe")          # PE bank chunks done
        s_act = sem("s_act")        # ACT psum->sbuf chunks done
        s_y = sem("s_y")            # yf ready
        s_out = sem("s_out")

        # ---------------- sync queue: t, g0 bounce+loads, hi_t, out ------
        @block.sync
        def _(s):
            s.dma_start(t_sb[:], t_d[:]).then_inc(s_in, 16)
            # u replicated+grouped via a DRAM bounce (group 0 on this queue)
            s.wait_ge(s_chain, 1)
            s.dma_start(uscr_d.ap()[0:GSZ], lo16[0:64, :]).then_inc(s_uscr, 16)
            s.wait_ge(s_uscr, 16)
            for (a0, a1) in LCH:
                src = uscr_d.ap()[a0:a1].unsqueeze(0).broadcast_to((64, a1 - a0))
                s.dma_start(u_rep[0:64, a0:a1], src).then_inc(s_urep, 16)
            s.wait_ge(s_out, 64)

        # ---------------- scalar queue: m2/bvec, g1 bounce+loads ----------
        @block.scalar
        def _(s):
            s.dma_start(m2[:], m2_d[:]).then_inc(s_in2, 16)
            s.dma_start(bvec[:], bv_d[:]).then_inc(s_in2, 16)
            s.wait_ge(s_chain, 1)
            s.dma_start(uscr_d.ap()[GSZ:2 * GSZ], lo16[64:128, :]
                        ).then_inc(s_uscr2, 16)
            s.wait_ge(s_uscr2, 16)
            for (a0, a1) in LCH:
                src = uscr_d.ap()[GSZ + a0:GSZ + a1] \
                    .unsqueeze(0).broadcast_to((64, a1 - a0))
                s.dma_start(u_rep[64:128, a0:a1], src).then_inc(s_urep2, 16)

        # ---------------- gpsimd: iota, oha, ohb chunk 2 ----------------
        @block.gpsimd
        def _(gp):
            gp.memset(ohb[:, 0:1024], 0.0).then_inc(s_warm, 1)
            gp.iota(iota_a[:], pattern=[[0, C], [1, A]], base=0,
                    channel_multiplier=0,
                    allow_small_or_imprecise_dtypes=True).then_inc(s_gp, 1)
            for hh, (p0, p1) in enumerate(MQS):
                gp.wait_ge(s_y, hh + 1)
                gp.dma_start(out_d.ap()[:, p0:p1, :],
                             yf[:, p0:p1, :]).then_inc(s_out, 16)

        # ---------------- vector: chain, one-hots, select ----------------
        @block.vector
        def _(v):
            v.wait_ge(s_in, 16)
            v.tensor_scalar(x[:], t_sb[:, 0:C], 1024.0, None, Alu.mult)
            v.tensor_scalar(iw[:], t_sb[:, 0:C], 1024.0, None, Alu.mult)
            v.tensor_copy(xf[:], iw[:])
            v.tensor_tensor(adj[:], xf[:], x[:], Alu.is_gt)
            v.tensor_tensor(u_i[:], iw[:], adj[:], Alu.subtract)
            v.tensor_scalar(lo32[:], u_i[:], 63, None, Alu.bitwise_and)
            v.tensor_copy(lo16[:], lo32[:]).then_inc(s_chain, 1)
            # hi from the host-transposed copy of t (cols C:2C)
            tT = t_sb[:, C:2 * C]
            v.tensor_scalar(x[:], tT, 1024.0, None, Alu.mult)
            v.tensor_scalar(iw[:], tT, 1024.0, None, Alu.mult)
            v.tensor_copy(xf[:], iw[:])
            v.tensor_tensor(adj[:], xf[:], x[:], Alu.is_gt)
            v.tensor_tensor(u_i[:], iw[:], adj[:], Alu.subtract)
            v.tensor_scalar(hi_i[:], u_i[:], 6, None, Alu.arith_shift_right)
            v.tensor_copy(hi_t[:], hi_i[:])
            # ohB q0, then ohA (fills the q1/q2 sem-wait bubble), then q1/q2
            v.wait_ge(s_in2, 32)
            def _ohb(q):
                a0, a1 = LCH[q]
                v.wait_ge(s_urep, 16 * (q + 1))
                v.wait_ge(s_urep2, 16 * (q + 1))
                v.tensor_scalar(ohb[:, a0:a1], u_rep[:, a0:a1],
                                bvec[:], None,
                                Alu.is_equal).then_inc(s_ohb, 1)
            # ohA on DVE (Pool rejects TensorTensor)
            v.wait_ge(s_gp, 1)
            hi_b = hi_t[:].rearrange("p (g jl) -> p jl g", g=NG) \
                .unsqueeze(3).broadcast_to((P, 64, NG, A))
            v.tensor_tensor(oha[:].rearrange("p (jl g) a -> p jl g a", g=NG),
                            hi_b, iota_a[:].rearrange(
                                "p (jl g) a -> p jl g a", g=NG),
                            Alu.is_equal)
            _ohb(0)
            _ohb(1)
            _ohb(2)
            _ohb(3)
            # select: multiply by ohA + tree-reduce in quarters for overlap
            for hh, (p0, p1) in enumerate(MQS):
                v.wait_ge(s_act, (p1 + 2 * BCH - 1) // (2 * BCH))
                sl = slice(p0, p1)
                # in1: oha broadcast over f (middle stride-0 dim)
                oha_b = oha[:, sl.start:sl.stop, :] \
                    .unsqueeze(2).broadcast_to((P, p1 - p0, F, A))
                v.tensor_tensor(psb[:, sl, :, :], psb[:, sl, :, :], oha_b,
                                Alu.mult)
                v.tensor_tensor(y0[:, sl], psb[:, sl, :, 0:8],
                                psb[:, sl, :, 8:16], Alu.add)
                v.tensor_tensor(y1[:, sl], y0[:, sl, :, 0:4],
                                y0[:, sl, :, 4:8], Alu.add)
                v.tensor_tensor(y2[:, sl], y1[:, sl, :, 0:2],
                                y1[:, sl, :, 2:4], Alu.add)
                v.tensor_tensor(yf[:, sl], y2[:, sl, :, 0],
                                y2[:, sl, :, 1], Alu.add).then_inc(s_y, 1)

        # ---------------- tensor: 64 paired matmuls ----------------
        # stationary = full ohb column slice (g0 rows 0-63, g1 rows 64-127),
        # moving = block-diag(M2g0, M2g1) -> out cols 0:48 = tile j (g0),
        # cols 48:96 = tile 64+j (g1).
        @block.tensor
        def _(te):
            # warm up the PE on zeroed ohb (overwritten later)
            te.wait_ge(s_warm, 1)
            te.matmul(psum[0][:, 0, 0:KW2], ohb[:, 0:128], ohb[:, 128:224],
                      start=True, stop=True, skip_group_check=True)
            te.matmul(psum[0][:, 1, 0:KW2], ohb[:, 0:128], ohb[:, 128:224],
                      start=True, stop=True, skip_group_check=True)
            te.wait_ge(s_in2, 16)   # m2 loaded
            for k in range(NCH):
                cmax = 128 * (BCH * k + BCH)
                qneed = next(qi + 1 for qi, (a0, a1) in enumerate(LCH)
                             if a1 >= cmax)
                te.wait_ge(s_ohb, qneed)
                if k >= 4:
                    te.wait_ge(s_act, k - 3)
                for j in range(BCH):
                    jl = BCH * k + j
                    lhsT = ohb[:, 128 * jl:128 * jl + 128]
                    out = psum[k % 4][:, j, 0:KW2]
                    mm = te.matmul(out, lhsT, m2[:], start=True, stop=True)
                    if j == BCH - 1:
                        mm.then_inc(s_pe, 1)

        # ---------------- scalar: PSUM -> SBUF fp16 copies ----------------
        # position pos2 = 2*jl + g; chunk k covers pos2 [8k, 8k+8) contiguous
        @block.scalar
        def _(sc):
            for k in range(NCH):
                if k % 2 == 0:
                    sc.wait_ge(s_pe, min(k + 2, NCH))
                dst = psb[:, 2 * BCH * k:2 * BCH * (k + 1), :, :] \
                    .rearrange("p (j g) f a -> p j (g f a)", j=BCH)
                sc.activation(
                    dst,
                    psum[k % 4][:, :, 0:KW2],
                    mybir.ActivationFunctionType.Copy,
                ).then_inc(s_act, 1)

    nc.compile()
    return nc


def _host_inputs(t, lut):
    tf0 = np.ascontiguousarray(np.asarray(t, np.float32)).reshape(
        N_CORES, P, C)
    tf = np.concatenate([tf0, tf0.transpose(0, 2, 1)], axis=2)
    m2 = np.zeros((P, KW2), np.float16)
    for g in range(NG):
        for b in range(Bw):
            for f in range(F):
                for a in range(A):
                    m2[64 * g + b, 48 * g + 16 * f + a] = lut[64 * a + b, f]
    bvec = (np.arange(P, dtype=np.float32) % 64).reshape(P, 1)
    return tf, m2, bvec


def kernel(t, W1, b1, W2, b2, W3, b3):
    global LAST_RESULTS
    if "nc" not in _CACHE:
        _CACHE["nc"] = _build_nc()
    nc = _CACHE["nc"]

    lut = _build_lut(np.asarray(W1, np.float32), np.asarray(b1, np.float32),
                     np.asarray(W2, np.float32), np.asarray(b2, np.float32),
                     np.asarray(W3, np.float32), np.asarray(b3, np.float32))
    tf, m2, bvec = _host_inputs(t, lut)
    in_maps = [{"t": np.ascontiguousarray(tf[m]), "m2": m2, "bvec": bvec}
               for m in range(N_CORES)]

    res = run_bass_kernel_spmd(nc, in_maps, list(range(N_CORES)), **RUN_KWARGS)
    LAST_RESULTS = res
    # out_d[c, pos, f]: pos -> source partition ORDER[pos]; element
    # e = p*128 + c  ->  out_d[c, inv_order[p], f]
    inv = np.argsort(np.asarray(ORDER))
    outs = []
    for m in range(N_CORES):
        o = res.results[m]["out"]            # [c=128, pos=128, 3]
        o = o[:, inv, :]                     # [c, p, 3]
        o = o.transpose(1, 0, 2).reshape(NPC, F)   # [e = p*128+c, 3]
        outs.append(o)
    return np.concatenate(outs, axis=0).reshape(B, T, F).astype(np.float32)
